# revision 6
# baseline (speedup 1.0000x reference)
"""GRU kernel for Trainium2, 8 NeuronCores, data-parallel over batch.

Strategy
--------
reference:  per step t (T=512):
    gi = [h, x_t]; r = sig(gi@Wr+br); z = sig(gi@Wz+bz)
    hh = tanh([h*r, x_t]@Wl+bl); h = (1-z)h + z*hh; out_t = relu(h@Wo+bo)

Decomposition per core (B_local=8 rows):
  Phase 1 (parallel over all t): XgT = Wx_g^T @ x^T + b_g for g in {r,z,l}
     (f32r matmuls, N=512) -> DRAM as bf16, transposed layout [H, B_local*T].
  Recurrence (serial, fully transposed domain; state hT [128 part, 8 chunks*8b]):
     per step, each gate's PSUM is seeded with its precomputed x-part via an
     identity matmul (all four seeds hoisted to the step start: h-independent
     PE work that fills the stall while h(t-1) finishes); the Wh^T h^T
     contraction accumulates on top (bf16 stationary weights resident in
     SBUF, LDW+MM pairs ~27-34ns) and ACT reads preacts straight from PSUM.
     The candidate gate runs in two PSUM halves so tanh/update of half A
     overlap half B's matmuls; (1-z)*h is precomputed during the l-gate so
     the critical tail after tanh is mul+add, written as bf16 history
     directly (separate f32 add keeps exact state). The output projection
     of block b-1 is spread over block b (compaction copies on steps 0-1,
     one 4-matmul half-group per step) to fill every step's residual stall.
  Host: pre-transposes x per core, un-permutes outT blocks.

Measured (8 cores, T=512): 4.327 ms HW exec, rel err 2.8e-3
(staged baseline: 5.254 ms).
"""
import os
import numpy as np
from contextlib import ExitStack

import concourse.bass as bass
import concourse.tile as tile
from concourse import bacc, mybir
from concourse import bass_utils

B, T_FULL, D, H = 64, 512, 1024, 1024
NCORES = 8
BL = B // NCORES            # 8 batch rows per core
KC = H // 128               # 8 contraction chunks
JC = H // 128               # 8 output chunks
BLK = 16                    # recurrence steps per output-projection block

f32 = mybir.dt.float32
f32r = mybir.dt.float32r
bf16 = mybir.dt.bfloat16
AF = mybir.ActivationFunctionType

_CACHE = {}


def build_program(T):
    cols = BL * T           # columns of the transposed activations
    nblk = T // BLK
    assert T % BLK == 0

    nc = bacc.Bacc("TRN2", target_bir_lowering=False, debug=False, num_devices=1)

    xT = nc.dram_tensor("xT", (H, cols), f32, kind="ExternalInput").ap()
    wx = {g: nc.dram_tensor(f"wx{g}", (D, H), f32, kind="ExternalInput").ap()
          for g in "rzl"}
    wh = {g: nc.dram_tensor(f"wh{g}", (H, H), f32, kind="ExternalInput").ap()
          for g in "rzl"}
    bias = {g: nc.dram_tensor(f"b{g}", (H, 1), f32, kind="ExternalInput").ap()
            for g in "rzl"}
    wo_d = nc.dram_tensor("wo", (H, H), f32, kind="ExternalInput").ap()
    bo_d = nc.dram_tensor("bo", (H, 1), f32, kind="ExternalInput").ap()
    id_d = nc.dram_tensor("ident", (128, 128), bf16, kind="ExternalInput").ap()
    outT = nc.dram_tensor("outT", (128, nblk * JC * BLK * BL), f32,
                          kind="ExternalOutput").ap()

    with tile.TileContext(nc) as tc, ExitStack() as top:
        dram = top.enter_context(tc.tile_pool(name="dram", bufs=1, space="DRAM"))
        xg_d = {g: dram.tile([H, cols], bf16, tag=f"X{g}", name=f"X{g}") for g in "rzl"}

        # ---------------- Phase 1: x projections (f32r) ----------------
        with ExitStack() as ctx:
            wp = ctx.enter_context(tc.tile_pool(name="p1w", bufs=1))
            xp = ctx.enter_context(tc.tile_pool(name="p1x", bufs=2))
            pp = ctx.enter_context(tc.tile_pool(name="p1ps", bufs=4, space="PSUM"))
            op = ctx.enter_context(tc.tile_pool(name="p1o", bufs=3))
            bp = ctx.enter_context(tc.tile_pool(name="p1b", bufs=1))

            wx_sb = {}
            bt = {}
            for g in "rzl":
                wx_sb[g] = wp.tile([128, KC * H], f32r, tag=f"wx{g}", name=f"wx{g}sb")
                for kc in range(KC):
                    nc.sync.dma_start(
                        wx_sb[g][:, kc * H:(kc + 1) * H],
                        wx[g][kc * 128:(kc + 1) * 128, :].bitcast(f32r))
                bt[g] = bp.tile([128, JC], f32, tag=f"b{g}", name=f"bt{g}")
                for jc in range(JC):
                    nc.sync.dma_start(bt[g][:, jc:jc + 1],
                                      bias[g][jc * 128:(jc + 1) * 128, :])

            NCB = 512
            for cb in range(cols // NCB):
                xt = xp.tile([128, KC * NCB], f32r, tag="xt")
                for kc in range(KC):
                    nc.sync.dma_start(
                        xt[:, kc * NCB:(kc + 1) * NCB],
                        xT[kc * 128:(kc + 1) * 128,
                           cb * NCB:(cb + 1) * NCB].bitcast(f32r))
                for g in "rzl":
                    for jc in range(JC):
                        ps = pp.tile([128, NCB], f32, tag="ps")
                        for kc in range(KC):
                            nc.tensor.matmul(
                                ps[:],
                                lhsT=wx_sb[g][:, kc * H + jc * 128:
                                              kc * H + (jc + 1) * 128],
                                rhs=xt[:, kc * NCB:(kc + 1) * NCB],
                                start=(kc == 0), stop=(kc == KC - 1))
                        ot = op.tile([128, NCB], bf16, tag="ot")
                        nc.scalar.activation(ot[:], ps[:], AF.Identity,
                                             bias=bt[g][:, jc:jc + 1])
                        nc.sync.dma_start(
                            xg_d[g][jc * 128:(jc + 1) * 128,
                                    cb * NCB:(cb + 1) * NCB], ot[:])

        # Phase-1 writes X* to DRAM via DMA; DRAM-tile RAW deps are not
        # reliably tracked by the scheduler, so fence before consuming.
        tc.strict_bb_all_engine_barrier()

        # ------------- Recurrence + fused output projection -------------
        with ExitStack() as ctx:
            wp = ctx.enter_context(tc.tile_pool(name="rw", bufs=1))
            sg = ctx.enter_context(tc.tile_pool(name="stg", bufs=2))
            xb = ctx.enter_context(tc.tile_pool(name="xblk", bufs=2))
            hi = ctx.enter_context(tc.tile_pool(name="hist", bufs=2))
            st = ctx.enter_context(tc.tile_pool(name="state", bufs=2))
            el = ctx.enter_context(tc.tile_pool(name="elt", bufs=2))
            pg = ctx.enter_context(tc.tile_pool(name="psg", bufs=1, space="PSUM"))
            p3 = ctx.enter_context(tc.tile_pool(name="ps3", bufs=2, space="PSUM"))
            o3 = ctx.enter_context(tc.tile_pool(name="o3", bufs=3))
            bp = ctx.enter_context(tc.tile_pool(name="rb", bufs=1))

            # resident bf16 weights (staged through f32)
            wh_sb = {}
            for g in "rzl":
                wh_sb[g] = wp.tile([128, KC * H], bf16, tag=f"wh{g}", name=f"wh{g}sb")
                for kc in range(KC):
                    stg = sg.tile([128, H], f32, tag="stg")
                    nc.sync.dma_start(stg[:], wh[g][kc * 128:(kc + 1) * 128, :])
                    nc.vector.tensor_copy(wh_sb[g][:, kc * H:(kc + 1) * H], stg[:])
            wo_sb = wp.tile([128, KC * H], bf16, tag="wo")
            for kc in range(KC):
                stg = sg.tile([128, H], f32, tag="stg")
                nc.sync.dma_start(stg[:], wo_d[kc * 128:(kc + 1) * 128, :])
                nc.vector.tensor_copy(wo_sb[:, kc * H:(kc + 1) * H], stg[:])
            bo_t = bp.tile([128, JC], f32, tag="bo")
            for jc in range(JC):
                nc.sync.dma_start(bo_t[:, jc:jc + 1],
                                  bo_d[jc * 128:(jc + 1) * 128, :])
            ident = wp.tile([128, 128], bf16, tag="id")
            nc.sync.dma_start(ident[:], id_d)

            CW = BL * KC        # 64: columns of a state tile (chunk-major, b minor)
            HW = CW // 2        # half width (j-chunks 0-3 / 4-7)
            hT = st.tile([128, CW], f32, tag="hT")
            nc.vector.memset(hT[:], 0.0)
            hz = bp.tile([128, CW], bf16, tag="h0")
            nc.vector.memset(hz[:], 0.0)
            hprev_src, hprev_off = hz, 0       # bf16 h^T of previous step

            def seed_mm(ps, xg3):
                # psum seeded by the x-part via an identity matmul
                # (start=True sets has_written for the whole range); the
                # recurrent contraction then accumulates on top. Seeds are
                # h-independent, so all four are hoisted to the step start
                # to fill the PE stall while h(t-1) finishes.
                nc.tensor.matmul(ps[:], lhsT=ident[:], rhs=xg3,
                                 start=True, stop=False)

            def acc_mm(ps, wt, src, off, jlo, jhi):
                njc = jhi - jlo
                for j in range(njc):
                    jc = jlo + j
                    for kc in range(KC):
                        nc.tensor.matmul(
                            ps[:, j * BL:(j + 1) * BL],
                            lhsT=wt[:, (kc * JC + jc) * 128:
                                    (kc * JC + jc + 1) * 128],
                            rhs=src[:, off + kc * BL:off + (kc + 1) * BL],
                            start=False,
                            stop=(j == njc - 1 and kc == KC - 1))

            def op_compact(hcmp, hist_src, kc):
                hv = hist_src[:].rearrange("p (t c b) -> p t c b", t=BLK, c=KC)
                nc.vector.tensor_copy(
                    hcmp[:, kc * BLK * BL:(kc + 1) * BLK * BL]
                    .rearrange("p (t b) -> p t b", t=BLK),
                    hv[:, :, kc, :])

            def op_half(hcmp, pso, jc, half):
                for k in range(KC // 2):
                    kc = half * (KC // 2) + k
                    nc.tensor.matmul(
                        pso[:],
                        lhsT=wo_sb[:, (kc * JC + jc) * 128:
                                   (kc * JC + jc + 1) * 128],
                        rhs=hcmp[:, kc * BLK * BL:(kc + 1) * BLK * BL],
                        start=(half == 0 and k == 0),
                        stop=(half == 1 and k == KC // 2 - 1))

            def op_finish(pbi, pso, jc):
                ou = o3.tile([128, BLK * BL], f32, tag="ou")
                nc.scalar.activation(ou[:], pso[:], AF.Relu,
                                     bias=bo_t[:, jc:jc + 1])
                nc.sync.dma_start(
                    outT[:, (pbi * JC + jc) * BLK * BL:
                         (pbi * JC + jc + 1) * BLK * BL], ou[:])

            def op_group(hcmp, pbi, jc):
                pso = p3.tile([128, BLK * BL], f32, tag="pso")
                op_half(hcmp, pso, jc, 0)
                op_half(hcmp, pso, jc, 1)
                op_finish(pbi, pso, jc)

            hist_prev = None
            for bi in range(nblk):
                xblk = {}
                for g in "rzl":
                    xblk[g] = xb.tile([128, KC * BLK * BL], bf16, tag=f"xb{g}", name=f"xb{g}t")
                    for kc in range(KC):
                        nc.sync.dma_start(
                            xblk[g][:, kc * BLK * BL:(kc + 1) * BLK * BL],
                            xg_d[g][kc * 128:(kc + 1) * 128,
                                    bi * BLK * BL:(bi + 1) * BLK * BL])
                hist = hi.tile([128, BLK * CW], bf16, tag="hist")
                if bi >= 1:
                    hcmp = o3.tile([128, KC * BLK * BL], bf16, tag="hcmp",
                                   name="hcmp")

                for dt in range(BLK):
                    def xsl(g, clo, chi):
                        return (xblk[g][:].rearrange("p (c s) -> p c s", c=KC)
                                [:, clo:chi, dt * BL:(dt + 1) * BL])
                    # all four x-seeds first: h-independent PE work that
                    # runs while the previous step's h-update finishes
                    psr = pg.tile([128, CW], f32, tag="gr")
                    seed_mm(psr, xsl("r", 0, KC))
                    psz = pg.tile([128, CW], f32, tag="gz")
                    seed_mm(psz, xsl("z", 0, KC))
                    psl_a = pg.tile([128, HW], f32, tag="gla")
                    seed_mm(psl_a, xsl("l", 0, 4))
                    psl_b = pg.tile([128, HW], f32, tag="glb")
                    seed_mm(psl_b, xsl("l", 4, 8))

                    acc_mm(psr, wh_sb["r"], hprev_src, hprev_off, 0, JC)
                    acc_mm(psz, wh_sb["z"], hprev_src, hprev_off, 0, JC)

                    r = el.tile([128, CW], f32, tag="r")
                    nc.scalar.activation(r[:], psr[:], AF.Sigmoid)
                    rh = el.tile([128, CW], bf16, tag="rh")
                    nc.vector.tensor_mul(rh[:], r[:], hT[:])
                    z = el.tile([128, CW], f32, tag="z")
                    nc.scalar.activation(z[:], psz[:], AF.Sigmoid)

                    acc_mm(psl_a, wh_sb["l"], rh, 0, 0, 4)
                    hh_a = el.tile([128, HW], f32, tag="hha")
                    nc.scalar.activation(hh_a[:], psl_a[:], AF.Tanh)

                    # (1-z)*h during the l-gate matmuls, so the critical
                    # tail after tanh is just mul + add
                    zh = el.tile([128, CW], f32, tag="zh")
                    nc.vector.tensor_mul(zh[:], z[:], hT[:])
                    az = el.tile([128, CW], f32, tag="az")
                    nc.vector.tensor_sub(az[:], hT[:], zh[:])

                    acc_mm(psl_b, wh_sb["l"], rh, 0, 4, 8)
                    hh_b = el.tile([128, HW], f32, tag="hhb")
                    nc.scalar.activation(hh_b[:], psl_b[:], AF.Tanh)

                    hTn = st.tile([128, CW], f32, tag="hT")
                    for half, hh in ((0, hh_a), (1, hh_b)):
                        lo, hi_ = half * HW, (half + 1) * HW
                        mm_ = el.tile([128, HW], f32, tag=f"m{half}")
                        nc.vector.tensor_mul(mm_[:], z[:, lo:hi_], hh[:])
                        nc.vector.tensor_add(
                            hist[:, dt * CW + lo:dt * CW + hi_],
                            az[:, lo:hi_], mm_[:])
                        nc.vector.tensor_add(hTn[:, lo:hi_],
                                             az[:, lo:hi_], mm_[:])
                    hT = hTn
                    hprev_src, hprev_off = hist, dt * CW

                    # interleaved output projection for the previous block:
                    # compaction on steps 0-1, then one 4-MM half-group per
                    # step so the fill reaches every step's stall.
                    if bi >= 1:
                        if dt < 2:
                            for q in range(4):
                                op_compact(hcmp, hist_prev, dt * 4 + q)
                        jc, half = divmod(dt, 2)
                        if half == 0:
                            pso_cur = p3.tile([128, BLK * BL], f32,
                                              tag="pso")
                            op_half(hcmp, pso_cur, jc, 0)
                        else:
                            op_half(hcmp, pso_cur, jc, 1)
                            op_finish(bi - 1, pso_cur, jc)
                hist_prev = hist

            # last block's projection (block-end form)
            hcmp = o3.tile([128, KC * BLK * BL], bf16, tag="hcmp", name="hcmp")
            for kc in range(KC):
                op_compact(hcmp, hist_prev, kc)
            for jc in range(JC):
                op_group(hcmp, nblk - 1, jc)

    nc.compile()
    return nc


def get_program(T):
    if T not in _CACHE:
        _CACHE[T] = build_program(T)
    return _CACHE[T]


def _prep_inputs(input, Wr, br, Wz, bz, Wl, bl, Wo, bo, Tt):
    import ml_dtypes
    cols = BL * Tt
    w_common = {
        "wxr": np.ascontiguousarray(Wr[H:]), "whr": np.ascontiguousarray(Wr[:H]),
        "wxz": np.ascontiguousarray(Wz[H:]), "whz": np.ascontiguousarray(Wz[:H]),
        "wxl": np.ascontiguousarray(Wl[H:]), "whl": np.ascontiguousarray(Wl[:H]),
        "br": np.ascontiguousarray(np.asarray(br).reshape(H, 1)),
        "bz": np.ascontiguousarray(np.asarray(bz).reshape(H, 1)),
        "bl": np.ascontiguousarray(np.asarray(bl).reshape(H, 1)),
        "wo": np.ascontiguousarray(Wo),
        "bo": np.ascontiguousarray(np.asarray(bo).reshape(H, 1)),
        "ident": np.eye(128, dtype=np.float32).astype(ml_dtypes.bfloat16),
    }
    in_maps = []
    for c in range(NCORES):
        xl = np.asarray(input[c * BL:(c + 1) * BL], dtype=np.float32)
        xTl = np.ascontiguousarray(xl.transpose(2, 1, 0).reshape(H, cols))
        in_maps.append({"xT": xTl, **w_common})
    return in_maps


def kernel(input, Wr, br, Wz, bz, Wl, bl, Wo, bo):
    Tt = input.shape[1]
    prog = get_program(Tt)
    in_maps = _prep_inputs(input, Wr, br, Wz, bz, Wl, bl, Wo, bo, Tt)

    res = bass_utils.run_bass_kernel_spmd(prog, in_maps,
                                          core_ids=list(range(NCORES)))
    nblk = Tt // BLK
    outs = []
    for c in range(NCORES):
        oT = res.results[c]["outT"]              # [128, nblk*JC*BLK*BL]
        o = oT.reshape(128, nblk, JC, BLK, BL)   # p, bi, j, dt, b
        o = o.transpose(4, 1, 3, 2, 0).reshape(BL, Tt, H)
        outs.append(o)
    return np.ascontiguousarray(np.concatenate(outs, axis=0))

